# revision 55
# baseline (speedup 1.0000x reference)
"""Trainium2 Bass kernel for nn_Att_AdaIn (B=4, C=256, H=W=64 attention block).

Sharding: 8 cores = 4 batches x 2 query-halves. Each core holds the fused
weights, the full key/value source y[b] ([256, 4096]), and its own query
slice x[b][:, half] ([256, 2048]); it computes the full attention output for
its 2048 queries. Host gathers the 8 [256, 2048] results.

Weight fusion done on the host (in float64):
  logits: S = k^T q with q = Wq x + bq, k = Wk y + bk
        = y^T (Wk^T Wq) x + y^T (Wk^T bq) 1^T + [per-query-constant terms]
    The per-query-constant (l-only) terms are softmax-invariant and dropped.
    So with  M = Wk^T Wq  and  bw = Wk^T bq:   qm = M x + bw,
    ST[j,l] = sum_c y[c,j] qm[c,l].
  output: Wo (V E / den) + bo  with V = Wv y + bv 1^T
        = (Wo Wv) y E / den + Wo bv + bo
    So with MoT = (Wo Wv)^T and xres' = x + (bo + Wo bv), the value
    projection vTo = y^T MoT directly produces Wo-mixed values.

All matmuls run fp8(e4m3) DoubleRow. The softmax exp is split between two
engines so neither is a bottleneck:
  - ACT chunks: true exp via the activation LUT, output fp8.
  - DVE chunks: Schraudolph bit-trick exp. The qm projection is pre-scaled
    by A*SCALE with A = 8/ln2, so the score PSUM holds A*logits. One
    tensor_scalar computes int8(max(psum + B, 0)); the int8 bit pattern
    reinterpreted as fp8e4m3 is 2^((v-56)/8) ~ exp(logits)*2^((B-56)/8).
    B is chosen so the int value stays below 120 (fp8 Inf) for the max
    logit, and the ACT chunks apply the matching bias ln2*(B-56)/8 so both
    engines produce identically-scaled E. The scale cancels in softmax.

Per-core pipeline (layouts chosen so no on-chip transpose is needed):
  qm  = A*SCALE*(M x + bw)     [c, l]      (c on partitions)
  vTo = y^T MoT                [j, 256]    (j on partitions; interleaved
                                           into the first l-tile pass)
  ST  = y^T qm                 [j, l]      A * transposed logits
  E   = exp-ish(ST)            fp8, via ACT or DVE per key-chunk
  zq  = vTo^T E                [256, l]    unnormalized Wo-mixed output
  den = 1^T E                  [l]         softmax denominators (fp8 DR
                                           ones-matmul per chunk)
  out = zq * (1/den) + xres'   (xres' = x + bo2, folded on host)
"""

import os
import sys

for _p in ("/root/.axon_site", "/root/.axon_site/_ro/trn_rl_repo", "/opt/trn_rl_repo"):
    if os.path.isdir(_p) and _p not in sys.path:
        sys.path.append(_p)

import numpy as np

import concourse.bass as bass
from concourse import bacc, mybir, tile
from concourse import bass_utils

B, C, H, W = 4, 256, 64, 64
N = H * W          # 4096 pixels
NQ = N // 2        # 2048 queries per core
P = 128
A = C // P         # 2 channel chunks
LT = 512           # l-tile (query) width
NLT = NQ // LT     # 4 l-tiles
JC = N // P        # 32 key chunks
JP = JC // 2       # 16 key-chunk pairs (256 keys each)
SCALE = 1.0 / np.sqrt(np.float32(C))  # 1/16

A8 = 8.0 / np.log(2.0)            # fp8e4m3 Schraudolph exponent scale
B_TOP = 26.0                      # int8 affine bias (keeps max < 120 = Inf)
DVE_OFF = -0.44                   # centers the mantissa-PWL sawtooth
ACT_BIAS = np.log(2.0) * (B_TOP - 56.0) / 8.0  # matching ACT exp shift

# which key-chunk pairs run exp on DVE (rest on ACT), per l-tile
_dve0 = os.environ.get("ATT_DVE0", "")
_dven = os.environ.get("ATT_DVEN", "1,3,5,7,9,11,13,15")
POPS_FIRST = int(os.environ.get("ATT_POPS_FIRST", "0"))
LAG = int(os.environ.get("ATT_LAG", "2"))
GPS_TAIL = int(os.environ.get("ATT_GPS_TAIL", "1"))
ZQ_COPY = int(os.environ.get("ATT_ZQCOPY", "1"))
WARM_MM = int(os.environ.get("ATT_WARM_MM", "0"))
VTO_PRE = int(os.environ.get("ATT_VTO_PRE", "4"))  # vTo pairs emitted in prologue


def _parse_set(s):
    s = s.strip()
    return frozenset(int(t) for t in s.split(",") if t != "")


DVE_SETS = [_parse_set(_dve0)] + [_parse_set(_dven)] * (NLT - 1)


def build_nc():
    f32 = mybir.dt.float32
    f8 = mybir.dt.float8e4
    i8 = mybir.dt.int8
    DR = mybir.MatmulPerfMode.DoubleRow
    Exp = mybir.ActivationFunctionType.Exp
    Copy = mybir.ActivationFunctionType.Copy

    nc = bacc.Bacc("TRN2", target_bir_lowering=False, debug=False)

    # pk packs [mT | moTa | bw (f32 as 4xf8) | pad 12 | x] along the free axis
    # so the whole prologue arrives in one DMA. Row stride stays 16-aligned
    # (DoubleRow AP steps must be multiples of 16).
    PKW = C + C + 16 + NQ
    XO = C + C + 16                   # x offset inside pk
    pk_d = nc.dram_tensor("pk", [C, PKW], f8, kind="ExternalInput").ap()
    y8_d = nc.dram_tensor("y8", [C, N], f8, kind="ExternalInput").ap()
    xres_d = nc.dram_tensor("xres", [C, NQ], f32, kind="ExternalInput").ap()
    out_d = nc.dram_tensor("out", [C, NQ], f32, kind="ExternalOutput").ap()
    out_r = out_d.rearrange("(a p) n -> p a n", p=P)

    with tile.TileContext(nc) as tc:
        with (
            tc.tile_pool(name="const", bufs=1) as const,
            tc.tile_pool(name="epool", bufs=8) as epool,
            tc.tile_pool(name="opool", bufs=4) as opool,
            tc.tile_pool(name="rpool", bufs=2) as rpool,
            tc.tile_pool(name="ps_st", bufs=2, space="PSUM") as ps_st,
            tc.tile_pool(name="ps_zq", bufs=1, space="PSUM") as ps_zq,
            tc.tile_pool(name="ps_small", bufs=1, space="PSUM") as ps_small,
            tc.tile_pool(name="ps_v", bufs=1, space="PSUM") as ps_v,
        ):
            # ---- persistent SBUF tensors ----
            pk_sb = const.tile([P, A, PKW], f8)
            mT_sb = pk_sb[:, :, 0:C]
            moTa_sb = pk_sb[:, :, C:2 * C]
            x_sb = pk_sb[:, :, XO:]
            y8_sb = const.tile([P, A, N], f8)
            ones_p2 = const.tile([P, 2, 16], f8)
            ones_row = const.tile([1, P], mybir.dt.bfloat16)
            shift_sb = const.tile([P, 1], f32)
            warm_sb = const.tile([P, 1], f32)
            qm_sb = const.tile([P, A, NQ], f8)
            vTo_sb = const.tile([P, JC, C], f8)
            xres_sb = const.tile([P, A, NQ], f32)

            # ---- constants + ACT exp-table warmup (runs during DMA wait) ----
            nc.vector.memset(shift_sb, float(ACT_BIAS))
            nc.scalar.activation(out=warm_sb, in_=shift_sb, func=Exp)
            nc.vector.memset(ones_p2, 1.0)
            nc.vector.memset(ones_row, 1.0)

            # ---- PE clock warmup: ~4us of dummy matmuls while DMAs stream,
            # so the HAM un-throttles (1.2 -> 2.4 GHz) before real work.
            warm_mv = epool.tile([P, 2, LT], f8, name="warm_mv")
            nc.vector.memset(warm_mv, 1.0)
            for w in range(WARM_MM):
                ps_w = ps_st.tile([P, 2, LT], f32, name="ps_w", tag="st")
                for h in range(2):
                    nc.tensor.matmul(
                        ps_w[0:16, h, :],
                        ones_p2[:, :, 0:16],
                        warm_mv,
                        start=True, stop=True, perf_mode=DR,
                    )

            # ---- loads, in order of first use ----
            y8r_ = y8_d.rearrange("(a p) k -> p a k", p=P)
            pkr_ = pk_d.rearrange("(a p) k -> p a k", p=P)
            nc.sync.dma_start(out=pk_sb[:, :, :XO + LT], in_=pkr_[:, :, :XO + LT])
            ycuts = [0, 512, 1536, 2560, 3584, N]
            for q in range(len(ycuts) - 1):
                nc.sync.dma_start(
                    out=y8_sb[:, :, ycuts[q]:ycuts[q + 1]],
                    in_=y8r_[:, :, ycuts[q]:ycuts[q + 1]],
                )
                if q == 0:
                    nc.sync.dma_start(
                        out=pk_sb[:, :, XO + LT:], in_=pkr_[:, :, XO + LT:]
                    )
            nc.sync.dma_start(
                out=xres_sb, in_=xres_d.rearrange("(a p) n -> p a n", p=P)
            )

            # ---- qm projection for one l-tile: PSUM pair -> DVE bias-cast ----
            def emit_qm(lt):
                lsl = slice(lt * LT, (lt + 1) * LT)
                ps_q = ps_st.tile([P, 2, LT], f32, name="ps_q", tag="st")
                for och in range(A):
                    nc.tensor.matmul(
                        ps_q[:, och, :],
                        mT_sb[:, :, och * P:(och + 1) * P],
                        x_sb[:, :, lsl],
                        start=True, stop=True, perf_mode=DR,
                    )
                for och in range(A):
                    nc.vector.tensor_scalar_add(
                        out=qm_sb[:, och, lsl],
                        in0=ps_q[:, och, :],
                        scalar1=pk_sb[:, och, 2 * C:2 * C + 4].bitcast(f32),
                    )

            # ---- vTo for one key-chunk pair: PSUM pair -> DVE cast ----
            def emit_vto(jp):
                psv = ps_v.tile([P, 2, C], f32, name="psv", tag="v")
                for h in range(2):
                    jc = jp * 2 + h
                    nc.tensor.matmul(
                        psv[:, h, :],
                        y8_sb[:, :, jc * P:(jc + 1) * P],
                        moTa_sb,
                        start=True, stop=True, perf_mode=DR,
                    )
                nc.vector.tensor_copy(
                    out=vTo_sb[:, jp * 2:jp * 2 + 2, :].rearrange("p h o -> p (h o)"),
                    in_=psv.rearrange("p h o -> p (h o)"),
                )

            for vp in range(VTO_PRE):
                emit_vto(vp)
            emit_qm(0)

            # ---- tail for middle l-tiles, emitted inline at den-complete.
            # ACT copies free the zq PSUM banks early (so the next tile's
            # accumulation can start), the 1/den chain stays tiny on DVE,
            # and the normalize/residual ops run on the otherwise-idle
            # GPSIMD so the DVE exp queue is never displaced.
            def emit_tail_inline(lt, zq, den):
                lsl = slice(lt * LT, (lt + 1) * LT)
                r_sb = rpool.tile([1, LT], f32, name="r_sb", tag="r")
                nc.vector.reciprocal_approx_fast(out=r_sb, in_=den)
                r_bf = rpool.tile([1, LT], mybir.dt.bfloat16, name="r_bf", tag="rbf")
                nc.vector.tensor_copy(out=r_bf, in_=r_sb)
                z_sbs = []
                if ZQ_COPY:
                    for och in range(A):
                        z_sb = opool.tile([P, LT], f32, name="z_sb")
                        nc.scalar.activation(out=z_sb, in_=zq[och], func=Copy)
                        z_sbs.append(z_sb)
                rbc_ps = ps_small.tile([P, LT], f32, name="rbc_ps", tag="sm")
                nc.tensor.matmul(rbc_ps, ones_row, r_bf, start=True, stop=True)
                rbc_sb = rpool.tile([P, LT], f32, name="rbc_sb", tag="rbc")
                nc.scalar.activation(out=rbc_sb, in_=rbc_ps, func=Copy)
                for och in range(A):
                    if GPS_TAIL and ZQ_COPY:
                        o_sb = z_sbs[och]
                        nc.gpsimd.tensor_tensor(
                            out=o_sb, in0=o_sb, in1=rbc_sb,
                            op=mybir.AluOpType.mult,
                        )
                        nc.gpsimd.tensor_tensor(
                            out=o_sb, in0=o_sb, in1=xres_sb[:, och, lsl],
                            op=mybir.AluOpType.add,
                        )
                    else:
                        o_sb = opool.tile([P, LT], f32, name="o_sb")
                        src = z_sbs[och] if ZQ_COPY else zq[och]
                        nc.vector.tensor_mul(out=o_sb, in0=src, in1=rbc_sb)
                        nc.vector.tensor_add(
                            out=o_sb, in0=o_sb, in1=xres_sb[:, och, lsl]
                        )
                    nc.sync.dma_start(out=out_r[:, och, lsl], in_=o_sb)

            # ---- final-tile tail: nothing competes afterwards, so free the
            # zq banks via ACT copies in parallel with the 1/den chain and
            # run the normalize steps back-to-back.
            def emit_tail_a(lt, zq, den, state):
                r_sb = rpool.tile([1, LT], f32, name="r_sb", tag="r")
                nc.vector.reciprocal_approx_fast(out=r_sb, in_=den)
                r_bf = rpool.tile([1, LT], mybir.dt.bfloat16, name="r_bf", tag="rbf")
                nc.vector.tensor_copy(out=r_bf, in_=r_sb)
                zq_sb = []
                for och in range(A):
                    z_sb = opool.tile([P, LT], f32, name="z_sb")
                    nc.scalar.activation(out=z_sb, in_=zq[och], func=Copy)
                    zq_sb.append(z_sb)
                state.update(lt=lt, r_bf=r_bf, zq_sb=zq_sb, step=0)

            def emit_tail_step(state):
                step = state["step"]
                state["step"] = step + 1
                lt = state["lt"]
                lsl = slice(lt * LT, (lt + 1) * LT)
                if step == 0:
                    rbc_ps = ps_small.tile([P, LT], f32, name="rbc_ps", tag="sm")
                    nc.tensor.matmul(
                        rbc_ps, ones_row, state["r_bf"], start=True, stop=True
                    )
                    rbc_sb = rpool.tile([P, LT], f32, name="rbc_sb", tag="rbc")
                    nc.vector.tensor_copy(out=rbc_sb, in_=rbc_ps)
                    state["rbc_sb"] = rbc_sb
                elif step in (1, 3):
                    och = (step - 1) // 2
                    o_sb = opool.tile([P, LT], f32, name="o_sb")
                    nc.vector.tensor_mul(
                        out=o_sb, in0=state["zq_sb"][och], in1=state["rbc_sb"]
                    )
                    state["o_sb"] = o_sb
                elif step in (2, 4):
                    och = (step - 2) // 2
                    o_sb = state["o_sb"]
                    nc.vector.tensor_add(
                        out=o_sb, in0=o_sb, in1=xres_sb[:, och, lsl]
                    )
                    nc.sync.dma_start(out=out_r[:, och, lsl], in_=o_sb)

            # ---- attention: one flat loop over (l-tile, key-chunk pair).
            # zq/den matmuls lag the ST/exp emission by LAG steps so the PE
            # (whose queue is in-order) never sits behind an in-flight exp.
            zq_cur = den_cur = None
            queue = []   # (lt, jp, zq pair, den, e8) awaiting zq/den emit
            tail_state = {"step": 99}

            def emit_pop():
                plt, pjp, pzq, pden, pe8 = queue.pop(0)
                for m in range(A):
                    nc.tensor.matmul(
                        pzq[m],
                        vTo_sb[:, pjp * 2:pjp * 2 + 2, m * P:(m + 1) * P],
                        pe8,
                        start=(pjp == 0),
                        stop=(pjp == JP - 1),
                        perf_mode=DR,
                    )
                nc.tensor.matmul(
                    pden,
                    ones_p2[:, :, 0:1],
                    pe8,
                    start=(pjp == 0),
                    stop=(pjp == JP - 1),
                    perf_mode=DR,
                )
                if pjp == JP - 1:
                    if plt < NLT - 1:
                        emit_tail_inline(plt, pzq, pden)
                    else:
                        emit_tail_a(plt, pzq, pden, tail_state)

            for gi in range(NLT * JP + LAG):
                lt, jp = divmod(gi, JP)
                if POPS_FIRST and gi >= LAG:
                    emit_pop()
                if gi < NLT * JP:
                    lsl = slice(lt * LT, (lt + 1) * LT)
                    if lt == 0:
                        if jp + VTO_PRE < JP:
                            emit_vto(jp + VTO_PRE)
                        if 1 <= jp <= 3:
                            emit_qm(jp)
                    if jp == 0:
                        zq_cur = (
                            ps_zq.tile([P, LT], f32, name="zq0", tag="zq0"),
                            ps_zq.tile([P, LT], f32, name="zq1", tag="zq1"),
                        )
                        den_cur = ps_small.tile([1, LT], f32, name="den", tag="sm")
                    st = ps_st.tile([P, 2, LT], f32, tag="st")
                    for h in range(2):
                        jc = jp * 2 + h
                        nc.tensor.matmul(
                            st[:, h, :],
                            y8_sb[:, :, jc * P:(jc + 1) * P],
                            qm_sb[:, :, lsl],
                            start=True, stop=True, perf_mode=DR,
                        )
                    e8 = epool.tile([P, 2, LT], f8)
                    if jp in DVE_SETS[lt]:
                        nc.vector.tensor_scalar(
                            out=e8.rearrange("p h l -> p (h l)").bitcast(i8),
                            in0=st.rearrange("p h l -> p (h l)"),
                            scalar1=float(B_TOP + DVE_OFF),
                            scalar2=0.0,
                            op0=mybir.AluOpType.add,
                            op1=mybir.AluOpType.max,
                        )
                    else:
                        nc.scalar.activation(
                            out=e8.rearrange("p h l -> p (h l)"),
                            in_=st.rearrange("p h l -> p (h l)"),
                            func=Exp,
                            scale=float(1.0 / A8),
                            bias=shift_sb,
                        )
                    queue.append((lt, jp, zq_cur, den_cur, e8))
                if not POPS_FIRST and gi >= LAG:
                    emit_pop()
                if tail_state["step"] <= 4:
                    emit_tail_step(tail_state)
            while tail_state["step"] <= 4:
                emit_tail_step(tail_state)

    nc.compile()
    return nc


_NC_CACHE = {}


def _get_nc(key="v2"):
    if key not in _NC_CACHE:
        _NC_CACHE[key] = build_nc()
    return _NC_CACHE[key]


def make_in_maps(x, y, Wq, bq, Wk, bk, Wv, bv, Wo, bo):
    import ml_dtypes

    f32 = np.float32
    f64 = np.float64
    f8np = ml_dtypes.float8_e4m3

    def to8(a):
        return np.clip(a, -240, 240).astype(f8np)

    xf = np.asarray(x, f32).reshape(B, C, N)
    yf = np.asarray(y, f32).reshape(B, C, N)
    Wq64, Wk64, Wv64, Wo64 = (np.asarray(w, f64) for w in (Wq, Wk, Wv, Wo))
    bq64, bv64, bo64 = (np.asarray(b, f64) for b in (bq, bv, bo))
    F = A8 * SCALE
    mT = to8(np.ascontiguousarray((Wk64.T @ Wq64).T) * F)
    moTa = to8(np.ascontiguousarray((Wo64 @ Wv64).T))
    bw = (F * (Wk64.T @ bq64)).astype(f32)
    bo2 = (bo64 + Wo64 @ bv64).astype(f32)
    bw8 = bw[:, None].view(np.uint8).view(f8np)        # [C, 4]
    pad = np.zeros((C, 12), f8np)
    in_maps = []
    y8s = [to8(yf[b]) for b in range(B)]
    for core in range(8):
        b, h = divmod(core, 2)
        xs = np.ascontiguousarray(xf[b][:, h * NQ:(h + 1) * NQ])
        pk = np.concatenate([mT, moTa, bw8, pad, to8(xs)], axis=1)
        in_maps.append({
            "pk": np.ascontiguousarray(pk),
            "y8": y8s[b],
            "xres": xs + bo2[:, None].astype(f32),
        })
    return in_maps


def kernel(x, y, Wq, bq, Wk, bk, Wv, bv, Wo, bo):
    import contextlib

    import jax

    nc = _get_nc()
    in_maps = make_in_maps(x, y, Wq, bq, Wk, bk, Wv, bv, Wo, bo)
    # Pin the axon (NeuronCore) backend: run_bass_via_pjrt uses jax.devices(),
    # which follows the ambient default platform and silently miscomputes if a
    # caller set the default to CPU.
    try:
        axon_devs = jax.devices("axon")
    except RuntimeError:
        axon_devs = None
    ctx = jax.default_device(axon_devs[0]) if axon_devs else contextlib.nullcontext()
    with ctx:
        res = bass_utils.run_bass_kernel_spmd(nc, in_maps, core_ids=list(range(8)))
    out = np.empty((B, C, N), np.float32)
    for core in range(8):
        b, h = divmod(core, 2)
        out[b][:, h * NQ:(h + 1) * NQ] = res.results[core]["out"]
    return out.reshape(B, C, H, W)


# revision 57
# speedup vs baseline: 1.1906x; 1.1906x over previous
"""Trainium2 Bass kernel for nn_Att_AdaIn (B=4, C=256, H=W=64 attention block).

Sharding: 8 cores = 4 batches x 2 query-halves. Each core holds the fused
weights, the full key/value source y[b] ([256, 4096]), and its own query
slice x[b][:, half] ([256, 2048]); it computes the full attention output for
its 2048 queries. Host gathers the 8 [256, 2048] results.

Weight fusion done on the host (in float64):
  logits: S = k^T q with q = Wq x + bq, k = Wk y + bk
        = y^T (Wk^T Wq) x + y^T (Wk^T bq) 1^T + [per-query-constant terms]
    The per-query-constant (l-only) terms are softmax-invariant and dropped.
    So with  M = Wk^T Wq  and  bw = Wk^T bq:   qm = M x + bw,
    ST[j,l] = sum_c y[c,j] qm[c,l].
  output: Wo (V E / den) + bo  with V = Wv y + bv 1^T
        = (Wo Wv) y E / den + Wo bv + bo
    So with MoT = (Wo Wv)^T and xres' = x + (bo + Wo bv), the value
    projection vTo = y^T MoT directly produces Wo-mixed values.

All matmuls run fp8(e4m3) DoubleRow. The softmax exp is split between two
engines so neither is a bottleneck:
  - ACT chunks: true exp via the activation LUT, output fp8.
  - DVE chunks: Schraudolph bit-trick exp. The qm projection is pre-scaled
    by A*SCALE with A = 8/ln2, so the score PSUM holds A*logits. One
    tensor_scalar computes int8(max(psum + B, 0)); the int8 bit pattern
    reinterpreted as fp8e4m3 is 2^((v-56)/8) ~ exp(logits)*2^((B-56)/8).
    B is chosen so the int value stays below 120 (fp8 Inf) for the max
    logit, and the ACT chunks apply the matching bias ln2*(B-56)/8 so both
    engines produce identically-scaled E. The scale cancels in softmax.

Per-core pipeline (layouts chosen so no on-chip transpose is needed):
  qm  = A*SCALE*(M x + bw)     [c, l]      (c on partitions)
  vTo = y^T MoT                [j, 256]    (j on partitions; interleaved
                                           into the first l-tile pass)
  ST  = y^T qm                 [j, l]      A * transposed logits
  E   = exp-ish(ST)            fp8, via ACT or DVE per key-chunk
  zq  = vTo^T E                [256, l]    unnormalized Wo-mixed output
  den = 1^T E                  [l]         softmax denominators (fp8 DR
                                           ones-matmul per chunk)
  out = zq * (1/den) + xres'   (xres' = x + bo2, folded on host)
"""

import os
import sys

for _p in ("/root/.axon_site", "/root/.axon_site/_ro/trn_rl_repo", "/opt/trn_rl_repo"):
    if os.path.isdir(_p) and _p not in sys.path:
        sys.path.append(_p)

import numpy as np

import concourse.bass as bass
from concourse import bacc, mybir, tile
from concourse import bass_utils

B, C, H, W = 4, 256, 64, 64
N = H * W          # 4096 pixels
NQ = N // 2        # 2048 queries per core
P = 128
A = C // P         # 2 channel chunks
LT = 512           # l-tile (query) width
NLT = NQ // LT     # 4 l-tiles
JC = N // P        # 32 key chunks
JP = JC // 2       # 16 key-chunk pairs (256 keys each)
SCALE = 1.0 / np.sqrt(np.float32(C))  # 1/16

A8 = 8.0 / np.log(2.0)            # fp8e4m3 Schraudolph exponent scale
B_TOP = 26.0                      # int8 affine bias (keeps max < 120 = Inf)
DVE_OFF = -0.44                   # centers the mantissa-PWL sawtooth
ACT_BIAS = np.log(2.0) * (B_TOP - 56.0) / 8.0  # matching ACT exp shift

# which key-chunk pairs run exp on DVE (rest on ACT), per l-tile
_dve0 = os.environ.get("ATT_DVE0", "")
_dven = os.environ.get("ATT_DVEN", "1,3,5,7,9,11,13,15")
POPS_FIRST = int(os.environ.get("ATT_POPS_FIRST", "0"))
LAG = int(os.environ.get("ATT_LAG", "2"))
GPS_TAIL = int(os.environ.get("ATT_GPS_TAIL", "1"))
ZQ_COPY = int(os.environ.get("ATT_ZQCOPY", "1"))
WARM_MM = int(os.environ.get("ATT_WARM_MM", "0"))
VTO_PRE = int(os.environ.get("ATT_VTO_PRE", "4"))  # vTo pairs emitted in prologue


def _parse_set(s):
    s = s.strip()
    return frozenset(int(t) for t in s.split(",") if t != "")


DVE_SETS = [_parse_set(_dve0)] + [_parse_set(_dven)] * (NLT - 1)


def build_nc():
    f32 = mybir.dt.float32
    f8 = mybir.dt.float8e4
    i8 = mybir.dt.int8
    DR = mybir.MatmulPerfMode.DoubleRow
    Exp = mybir.ActivationFunctionType.Exp
    Copy = mybir.ActivationFunctionType.Copy

    nc = bacc.Bacc("TRN2", target_bir_lowering=False, debug=False)

    # pk packs [mT | moTa | bw (f32 as 4xf8) | pad 12 | x] along the free axis
    # so the whole prologue arrives in one DMA. Row stride stays 16-aligned
    # (DoubleRow AP steps must be multiples of 16).
    PKW = C + C + 16 + NQ
    XO = C + C + 16                   # x offset inside pk
    pk_d = nc.dram_tensor("pk", [C, PKW], f8, kind="ExternalInput").ap()
    y8_d = nc.dram_tensor("y8", [C, N], f8, kind="ExternalInput").ap()
    xres_d = nc.dram_tensor("xres", [C, NQ], f32, kind="ExternalInput").ap()
    out_d = nc.dram_tensor("out", [C, NQ], f32, kind="ExternalOutput").ap()
    out_r = out_d.rearrange("(a p) n -> p a n", p=P)

    with tile.TileContext(nc) as tc:
        with (
            tc.tile_pool(name="const", bufs=1) as const,
            tc.tile_pool(name="epool", bufs=8) as epool,
            tc.tile_pool(name="opool", bufs=4) as opool,
            tc.tile_pool(name="rpool", bufs=2) as rpool,
            tc.tile_pool(name="ps_st", bufs=2, space="PSUM") as ps_st,
            tc.tile_pool(name="ps_zq", bufs=1, space="PSUM") as ps_zq,
            tc.tile_pool(name="ps_small", bufs=1, space="PSUM") as ps_small,
            tc.tile_pool(name="ps_v", bufs=1, space="PSUM") as ps_v,
        ):
            # ---- persistent SBUF tensors ----
            pk_sb = const.tile([P, A, PKW], f8)
            mT_sb = pk_sb[:, :, 0:C]
            moTa_sb = pk_sb[:, :, C:2 * C]
            x_sb = pk_sb[:, :, XO:]
            y8_sb = const.tile([P, A, N], f8)
            ones_p2 = const.tile([P, 2, 16], f8)
            ones_row = const.tile([1, P], mybir.dt.bfloat16)
            shift_sb = const.tile([P, 1], f32)
            warm_sb = const.tile([P, 1], f32)
            qm_sb = const.tile([P, A, NQ], f8)
            vTo_sb = const.tile([P, JC, C], f8)
            xres_sb = const.tile([P, A, NQ], f32)

            # ---- constants + ACT exp-table warmup (runs during DMA wait) ----
            nc.vector.memset(shift_sb, float(ACT_BIAS))
            nc.scalar.activation(out=warm_sb, in_=shift_sb, func=Exp)
            nc.vector.memset(ones_p2, 1.0)
            nc.vector.memset(ones_row, 1.0)

            # ---- PE clock warmup: ~4us of dummy matmuls while DMAs stream,
            # so the HAM un-throttles (1.2 -> 2.4 GHz) before real work.
            warm_mv = epool.tile([P, 2, LT], f8, name="warm_mv")
            nc.vector.memset(warm_mv, 1.0)
            for w in range(WARM_MM):
                ps_w = ps_st.tile([P, 2, LT], f32, name="ps_w", tag="st")
                for h in range(2):
                    nc.tensor.matmul(
                        ps_w[0:16, h, :],
                        ones_p2[:, :, 0:16],
                        warm_mv,
                        start=True, stop=True, perf_mode=DR,
                    )

            # ---- loads, in order of first use ----
            y8r_ = y8_d.rearrange("(a p) k -> p a k", p=P)
            pkr_ = pk_d.rearrange("(a p) k -> p a k", p=P)
            nc.sync.dma_start(out=pk_sb[:, :, :XO + LT], in_=pkr_[:, :, :XO + LT])
            ycuts = [0, 512, 1536, 2560, 3584, N]
            for q in range(len(ycuts) - 1):
                nc.sync.dma_start(
                    out=y8_sb[:, :, ycuts[q]:ycuts[q + 1]],
                    in_=y8r_[:, :, ycuts[q]:ycuts[q + 1]],
                )
                if q == 0:
                    nc.sync.dma_start(
                        out=pk_sb[:, :, XO + LT:], in_=pkr_[:, :, XO + LT:]
                    )
            nc.sync.dma_start(
                out=xres_sb, in_=xres_d.rearrange("(a p) n -> p a n", p=P)
            )

            # ---- qm projection for one l-tile: PSUM pair -> DVE bias-cast ----
            def emit_qm(lt):
                lsl = slice(lt * LT, (lt + 1) * LT)
                ps_q = ps_st.tile([P, 2, LT], f32, name="ps_q", tag="st")
                for och in range(A):
                    nc.tensor.matmul(
                        ps_q[:, och, :],
                        mT_sb[:, :, och * P:(och + 1) * P],
                        x_sb[:, :, lsl],
                        start=True, stop=True, perf_mode=DR,
                    )
                for och in range(A):
                    nc.vector.tensor_scalar_add(
                        out=qm_sb[:, och, lsl],
                        in0=ps_q[:, och, :],
                        scalar1=pk_sb[:, och, 2 * C:2 * C + 4].bitcast(f32),
                    )

            # ---- vTo for one key-chunk pair: PSUM pair -> DVE cast ----
            def emit_vto(jp):
                psv = ps_v.tile([P, 2, C], f32, name="psv", tag="v")
                for h in range(2):
                    jc = jp * 2 + h
                    nc.tensor.matmul(
                        psv[:, h, :],
                        y8_sb[:, :, jc * P:(jc + 1) * P],
                        moTa_sb,
                        start=True, stop=True, perf_mode=DR,
                    )
                nc.vector.tensor_copy(
                    out=vTo_sb[:, jp * 2:jp * 2 + 2, :].rearrange("p h o -> p (h o)"),
                    in_=psv.rearrange("p h o -> p (h o)"),
                )

            for vp in range(VTO_PRE):
                emit_vto(vp)
            emit_qm(0)

            # ---- tail for middle l-tiles, emitted inline at den-complete.
            # ACT copies free the zq PSUM banks early (so the next tile's
            # accumulation can start), the 1/den chain stays tiny on DVE,
            # and the normalize/residual ops run on the otherwise-idle
            # GPSIMD so the DVE exp queue is never displaced.
            def emit_tail_inline(lt, zq, den):
                lsl = slice(lt * LT, (lt + 1) * LT)
                r_sb = rpool.tile([1, LT], f32, name="r_sb", tag="r")
                nc.vector.reciprocal_approx_fast(out=r_sb, in_=den)
                r_bf = rpool.tile([1, LT], mybir.dt.bfloat16, name="r_bf", tag="rbf")
                nc.vector.tensor_copy(out=r_bf, in_=r_sb)
                z_sbs = []
                if ZQ_COPY:
                    for och in range(A):
                        z_sb = opool.tile([P, LT], f32, name="z_sb")
                        nc.scalar.activation(out=z_sb, in_=zq[och], func=Copy)
                        z_sbs.append(z_sb)
                rbc_ps = ps_small.tile([P, LT], f32, name="rbc_ps", tag="sm")
                nc.tensor.matmul(rbc_ps, ones_row, r_bf, start=True, stop=True)
                rbc_sb = rpool.tile([P, LT], f32, name="rbc_sb", tag="rbc")
                nc.scalar.activation(out=rbc_sb, in_=rbc_ps, func=Copy)
                for och in range(A):
                    if GPS_TAIL and ZQ_COPY:
                        o_sb = z_sbs[och]
                        nc.gpsimd.tensor_tensor(
                            out=o_sb, in0=o_sb, in1=rbc_sb,
                            op=mybir.AluOpType.mult,
                        )
                        nc.gpsimd.tensor_tensor(
                            out=o_sb, in0=o_sb, in1=xres_sb[:, och, lsl],
                            op=mybir.AluOpType.add,
                        )
                    else:
                        o_sb = opool.tile([P, LT], f32, name="o_sb")
                        src = z_sbs[och] if ZQ_COPY else zq[och]
                        nc.vector.tensor_mul(out=o_sb, in0=src, in1=rbc_sb)
                        nc.vector.tensor_add(
                            out=o_sb, in0=o_sb, in1=xres_sb[:, och, lsl]
                        )
                    nc.sync.dma_start(out=out_r[:, och, lsl], in_=o_sb)

            # ---- final-tile tail: nothing competes afterwards, so free the
            # zq banks via ACT copies in parallel with the 1/den chain and
            # run the normalize steps back-to-back.
            def emit_tail_a(lt, zq, den, state):
                r_sb = rpool.tile([1, LT], f32, name="r_sb", tag="r")
                nc.vector.reciprocal_approx_fast(out=r_sb, in_=den)
                r_bf = rpool.tile([1, LT], mybir.dt.bfloat16, name="r_bf", tag="rbf")
                nc.vector.tensor_copy(out=r_bf, in_=r_sb)
                zq_sb = []
                for och in range(A):
                    z_sb = opool.tile([P, LT], f32, name="z_sb")
                    nc.scalar.activation(out=z_sb, in_=zq[och], func=Copy)
                    zq_sb.append(z_sb)
                state.update(lt=lt, r_bf=r_bf, zq_sb=zq_sb, step=0)

            def emit_tail_step(state):
                step = state["step"]
                state["step"] = step + 1
                lt = state["lt"]
                lsl = slice(lt * LT, (lt + 1) * LT)
                if step == 0:
                    rbc_ps = ps_small.tile([P, LT], f32, name="rbc_ps", tag="sm")
                    nc.tensor.matmul(
                        rbc_ps, ones_row, state["r_bf"], start=True, stop=True
                    )
                    rbc_sb = rpool.tile([P, LT], f32, name="rbc_sb", tag="rbc")
                    nc.vector.tensor_copy(out=rbc_sb, in_=rbc_ps)
                    state["rbc_sb"] = rbc_sb
                elif step in (1, 3):
                    och = (step - 1) // 2
                    o_sb = opool.tile([P, LT], f32, name="o_sb")
                    nc.vector.tensor_mul(
                        out=o_sb, in0=state["zq_sb"][och], in1=state["rbc_sb"]
                    )
                    state["o_sb"] = o_sb
                elif step in (2, 4):
                    och = (step - 2) // 2
                    o_sb = state["o_sb"]
                    nc.vector.tensor_add(
                        out=o_sb, in0=o_sb, in1=xres_sb[:, och, lsl]
                    )
                    nc.sync.dma_start(out=out_r[:, och, lsl], in_=o_sb)

            # ---- attention: one flat loop over (l-tile, key-chunk pair).
            # zq/den matmuls lag the ST/exp emission by LAG steps so the PE
            # (whose queue is in-order) never sits behind an in-flight exp.
            zq_cur = den_cur = None
            queue = []   # (lt, jp, zq pair, den, e8) awaiting zq/den emit
            tail_state = {"step": 99}

            def emit_pop():
                plt, pjp, pzq, pden, pe8 = queue.pop(0)
                for m in range(A):
                    nc.tensor.matmul(
                        pzq[m],
                        vTo_sb[:, pjp * 2:pjp * 2 + 2, m * P:(m + 1) * P],
                        pe8,
                        start=(pjp == 0),
                        stop=(pjp == JP - 1),
                        perf_mode=DR,
                    )
                nc.tensor.matmul(
                    pden,
                    ones_p2[:, :, 0:1],
                    pe8,
                    start=(pjp == 0),
                    stop=(pjp == JP - 1),
                    perf_mode=DR,
                )
                if pjp == JP - 1:
                    if plt < NLT - 1:
                        emit_tail_inline(plt, pzq, pden)
                    else:
                        emit_tail_a(plt, pzq, pden, tail_state)

            for gi in range(NLT * JP + LAG):
                lt, jp = divmod(gi, JP)
                if POPS_FIRST and gi >= LAG:
                    emit_pop()
                if gi < NLT * JP:
                    lsl = slice(lt * LT, (lt + 1) * LT)
                    if lt == 0:
                        if jp + VTO_PRE < JP:
                            emit_vto(jp + VTO_PRE)
                        if 1 <= jp <= 3:
                            emit_qm(jp)
                    if jp == 0:
                        zq_cur = (
                            ps_zq.tile([P, LT], f32, name="zq0", tag="zq0"),
                            ps_zq.tile([P, LT], f32, name="zq1", tag="zq1"),
                        )
                        den_cur = ps_small.tile([1, LT], f32, name="den", tag="sm")
                    st = ps_st.tile([P, 2, LT], f32, tag="st")
                    for h in range(2):
                        jc = jp * 2 + h
                        nc.tensor.matmul(
                            st[:, h, :],
                            y8_sb[:, :, jc * P:(jc + 1) * P],
                            qm_sb[:, :, lsl],
                            start=True, stop=True, perf_mode=DR,
                        )
                    e8 = epool.tile([P, 2, LT], f8)
                    if jp in DVE_SETS[lt]:
                        nc.vector.tensor_scalar(
                            out=e8.rearrange("p h l -> p (h l)").bitcast(i8),
                            in0=st.rearrange("p h l -> p (h l)"),
                            scalar1=float(B_TOP + DVE_OFF),
                            scalar2=0.0,
                            op0=mybir.AluOpType.add,
                            op1=mybir.AluOpType.max,
                        )
                    else:
                        nc.scalar.activation(
                            out=e8.rearrange("p h l -> p (h l)"),
                            in_=st.rearrange("p h l -> p (h l)"),
                            func=Exp,
                            scale=float(1.0 / A8),
                            bias=shift_sb,
                        )
                    queue.append((lt, jp, zq_cur, den_cur, e8))
                if not POPS_FIRST and gi >= LAG:
                    emit_pop()
                if tail_state["step"] <= 4:
                    emit_tail_step(tail_state)
            while tail_state["step"] <= 4:
                emit_tail_step(tail_state)

    nc.compile()
    return nc


_NC_CACHE = {}


def _get_nc(key="v2"):
    if key not in _NC_CACHE:
        _NC_CACHE[key] = build_nc()
    return _NC_CACHE[key]


def make_in_maps(x, y, Wq, bq, Wk, bk, Wv, bv, Wo, bo):
    import ml_dtypes

    f32 = np.float32
    f64 = np.float64
    f8np = ml_dtypes.float8_e4m3

    def to8(a):
        return np.clip(a, -240, 240).astype(f8np)

    xf = np.asarray(x, f32).reshape(B, C, N)
    yf = np.asarray(y, f32).reshape(B, C, N)
    Wq64, Wk64, Wv64, Wo64 = (np.asarray(w, f64) for w in (Wq, Wk, Wv, Wo))
    bq64, bv64, bo64 = (np.asarray(b, f64) for b in (bq, bv, bo))
    F = A8 * SCALE
    mT = to8(np.ascontiguousarray((Wk64.T @ Wq64).T) * F)
    moTa = to8(np.ascontiguousarray((Wo64 @ Wv64).T))
    bw = (F * (Wk64.T @ bq64)).astype(f32)
    bo2 = (bo64 + Wo64 @ bv64).astype(f32)
    bw8 = bw[:, None].view(np.uint8).view(f8np)        # [C, 4]
    pad = np.zeros((C, 12), f8np)
    in_maps = []
    y8s = [to8(yf[b]) for b in range(B)]
    for core in range(8):
        b, h = divmod(core, 2)
        xs = np.ascontiguousarray(xf[b][:, h * NQ:(h + 1) * NQ])
        pk = np.concatenate([mT, moTa, bw8, pad, to8(xs)], axis=1)
        in_maps.append({
            "pk": np.ascontiguousarray(pk),
            "y8": y8s[b],
            "xres": xs + bo2[:, None].astype(f32),
        })
    return in_maps


def kernel(x, y, Wq, bq, Wk, bk, Wv, bv, Wo, bo):
    import contextlib

    import jax

    nc = _get_nc()
    in_maps = make_in_maps(x, y, Wq, bq, Wk, bk, Wv, bv, Wo, bo)
    # Pin the axon (NeuronCore) backend: run_bass_via_pjrt uses jax.devices(),
    # which follows the ambient default platform and silently miscomputes if a
    # caller set the default to CPU.
    try:
        axon_devs = jax.devices("axon")
    except RuntimeError:
        axon_devs = None
    ctx = jax.default_device(axon_devs[0]) if axon_devs else contextlib.nullcontext()
    with ctx:
        res = bass_utils.run_bass_kernel_spmd(nc, in_maps, core_ids=list(range(8)))
    out = np.empty((B, C, N), np.float32)
    for core in range(8):
        b, h = divmod(core, 2)
        out[b][:, h * NQ:(h + 1) * NQ] = res.results[core]["out"]
    return out.reshape(B, C, H, W)
